# revision 20
# baseline (speedup 1.0000x reference)
"""Trainium2 Bass kernel for nn_CrossAttentionNew (sparse_attention).

Strategy (per sharding hint): B1 split 4-per-core across 8 cores; v2 side
replicated. The reference computation is algebraically restructured so the
dominant per-pair [*,1024]x[1024,1024] projections collapse into tiny
Gram-matrix bilinear forms:

  weighted_v2 @ Wwq2.T = p2 @ (v2[b] @ Wwq2.T)           (associativity)
  sf1 = p2 G2[b] p2.T,  G2[b] = scale * v2q[b] v2k[b].T  ([32,32] per b)
  sf2 = p1 G1[a] p1.T,  G1[a] = scale * v1q[a] v1k[a].T  ([36,36] per a)
  score = (u1 C u2.T) / ((sqrt(u1 Gv2 u1)+eps)*(sqrt(u2 Gv1 u2)+eps))
  with C = v2[b] v1[a].T, Gv* Gram matrices, u1 = 1.P_f1.p2, u2 = 1.P_f2.p1
  (the mean factors cancel inside l2norm up to eps).

All D-contractions run in bf16 (validated 1.4e-3 max rel err); final
bilinears in fp32. All matmul operands/outputs live at base partition 0
(non-zero base partitions fault on this runtime); 32-row blocks are moved
with partition-shift SBUF-to-SBUF DMAs instead.
"""

import os
import numpy as np
import ml_dtypes

import concourse.bass as bass
import concourse.mybir as mybir
import concourse.tile as tile
from concourse import bacc
from concourse.bass_utils import run_bass_kernel_spmd
from concourse.masks import make_identity

F32 = mybir.dt.float32
BF16 = mybir.dt.bfloat16
I32 = mybir.dt.int32
AX = mybir.AxisListType.X
EXP = mybir.ActivationFunctionType.Exp
SQRT = mybir.ActivationFunctionType.Sqrt

D = 1024
B1, B2, R, T = 32, 32, 36, 32
KT = 8          # D / 128 contraction tiles
AP_ = 4         # a-values per core
NCORES = 8
SCALE = 1.0 / 32.0   # 1/sqrt(D)
NEG = -1e9
EPS = 1e-8

W_NAMES = ["k1", "q1", "wq1", "wk1", "k2", "q2", "wq2", "wk2"]


def _body(nc, tc, ctx, dram_in, dram_out):
    STOP = int(os.environ.get("BASS_STOP", "99"))
    v1t_d, v2t_d, wts_d, mb_d, iters_d = dram_in
    score_d = dram_out

    # ---- pools ----
    persist = ctx.enter_context(tc.tile_pool(name="persist", bufs=1))
    wpool = ctx.enter_context(tc.tile_pool(name="wpool", bufs=5))
    vqk = ctx.enter_context(tc.tile_pool(name="vqk", bufs=2))
    temps = ctx.enter_context(tc.tile_pool(name="temps", bufs=3))
    sums = ctx.enter_context(tc.tile_pool(name="sums", bufs=4))
    smalls = ctx.enter_context(tc.tile_pool(name="smalls", bufs=2))

    # PSUM budget: 8 banks. ps_big: 2 x 4KB (4 banks); ps_tv: 3 x 1 bank;
    # finals (tag tv_fin): 1 bank.
    ps_big = ctx.enter_context(tc.tile_pool(name="ps_big", bufs=2, space="PSUM"))
    ps_tv = ctx.enter_context(tc.tile_pool(name="ps_tv", bufs=3, space="PSUM"))

    # ---- persistent sbuf tiles ----
    v2t_sb = persist.tile([128, KT, B2 * T], BF16)
    v1t_sb = persist.tile([128, KT, AP_ * R], BF16)
    mb36 = persist.tile([R, B2 * T], F32)
    id128 = persist.tile([128, 128], BF16)
    ones32 = persist.tile([T, 1], F32)
    ones36 = persist.tile([R, 1], F32)

    q2t_sb = persist.tile([128, KT, B2 * T], BF16)
    k2t_sb = persist.tile([128, KT, B2 * T], BF16)
    k1t_sb = persist.tile([128, KT, AP_ * R], BF16)
    q1t_sb = persist.tile([128, KT, AP_ * R], BF16)
    v1qt_sb = persist.tile([128, KT, AP_ * R], BF16)
    v1kt_sb = persist.tile([128, KT, AP_ * R], BF16)

    g2sb = persist.tile([T, B2 * T], F32)      # G2 accumulator (f32)
    gv2sb = persist.tile([T, B2 * T], F32)     # Gv2 (f32, used directly)
    g2bf = persist.tile([T, B2 * T], BF16)     # G2 as bf16 matmul operand
    g1_sb = persist.tile([R, AP_, R], BF16)
    gv1_sb = persist.tile([R, AP_, R], F32)
    c_all = persist.tile([R, AP_, B2 * T], F32)
    p1_all = persist.tile([128, 8, AP_, R], BF16)
    p1v = persist.tile([T, 8, 4, AP_, R], BF16)   # [t', g, bj, a, r] at base 0

    itt = persist.tile([1, 1], I32)
    nc.sync.dma_start(out=itt, in_=iters_d)
    n_iters = nc.values_load(itt.to_broadcast((1, 1)), skip_runtime_bounds_check=True)

    wt_view = wts_d

    def _iter_body():
        # ============ constants / input loads ============
        nc.sync.dma_start(out=v2t_sb, in_=v2t_d)
        nc.sync.dma_start(out=v1t_sb, in_=v1t_d)
        nc.sync.dma_start(
            out=mb36,
            in_=bass.AP(tensor=mb_d.tensor, offset=mb_d.offset, ap=[[0, R]] + list(mb_d.ap)),
        )
        make_identity(nc, id128)
        nc.vector.memset(ones32, 1.0)
        nc.vector.memset(ones36, 1.0)
        if STOP <= 1:
            return

        # ============ Gv2 first: PE work that needs no weight DMAs ============
        for kt in range(KT):
            gpart = ps_big.tile([T, B2 * T], F32, tag="big")
            for b in range(B2):
                nc.tensor.matmul(
                    gpart[:, b * T : (b + 1) * T],
                    lhsT=v2t_sb[:, kt, b * T : (b + 1) * T],
                    rhs=v2t_sb[:, kt, b * T : (b + 1) * T],
                    start=True, stop=True,
                )
            if kt == 0:
                nc.vector.tensor_copy(out=gv2sb, in_=gpart)
            else:
                nc.vector.tensor_add(out=gv2sb, in0=gv2sb, in1=gpart)

        # ============ C^T (r-major) per a ============
        for a in range(AP_):
            cps = ps_big.tile([R, 1024], F32, tag="big")
            for nh in range(2):
                for kt in range(KT):
                    nc.tensor.matmul(
                        cps[:, nh * 512 : (nh + 1) * 512],
                        lhsT=v1t_sb[:, kt, a * R : (a + 1) * R],
                        rhs=v2t_sb[:, kt, nh * 512 : (nh + 1) * 512],
                        start=(kt == 0),
                        stop=(kt == KT - 1),
                    )
            nc.any.tensor_copy(out=c_all[:, a, :], in_=cps)
        if STOP <= 6:
            return

        # ============ v2-side projections: Q2T, K2T ============
        for name, dst, scl in (("q2", q2t_sb, 1.0), ("k2", k2t_sb, 1.0)):
            for j in range(KT):
                wtile = wpool.tile([128, KT, 128], BF16, tag="w")
                nc.sync.dma_start(out=wtile, in_=wt_view[name][j])
                pj = ps_big.tile([128, 1024], F32, tag="big")
                for nh in range(2):
                    for kt in range(KT):
                        nc.tensor.matmul(
                            pj[:, nh * 512 : (nh + 1) * 512],
                            lhsT=wtile[:, kt, :],
                            rhs=v2t_sb[:, kt, nh * 512 : (nh + 1) * 512],
                            start=(kt == 0),
                            stop=(kt == KT - 1),
                        )
                if scl == 1.0:
                    nc.vector.tensor_copy(out=dst[:, j, :], in_=pj)
                else:
                    nc.scalar.mul(out=dst[:, j, :], in_=pj, mul=scl)
        if STOP <= 2:
            return

        # ============ G2, Gv2 (SBUF-accumulated) ============
        for j in range(KT):
            vq_sb = vqk.tile([128, 1024], BF16, tag="vq")
            vk_sb = vqk.tile([128, 1024], BF16, tag="vk")
            for name, dst, scl in (("wq2", vq_sb, 1.0), ("wk2", vk_sb, 1.0)):
                wtile = wpool.tile([128, KT, 128], BF16, tag="w")
                nc.sync.dma_start(out=wtile, in_=wt_view[name][j])
                pj = ps_big.tile([128, 1024], F32, tag="big")
                for nh in range(2):
                    for kt in range(KT):
                        nc.tensor.matmul(
                            pj[:, nh * 512 : (nh + 1) * 512],
                            lhsT=wtile[:, kt, :],
                            rhs=v2t_sb[:, kt, nh * 512 : (nh + 1) * 512],
                            start=(kt == 0),
                            stop=(kt == KT - 1),
                        )
                if scl == 1.0:
                    nc.vector.tensor_copy(out=dst, in_=pj)
                else:
                    nc.scalar.mul(out=dst, in_=pj, mul=scl)
            gpart = ps_big.tile([T, B2 * T], F32, tag="big")
            for b in range(B2):
                nc.tensor.matmul(
                    gpart[:, b * T : (b + 1) * T],
                    lhsT=vq_sb[:, b * T : (b + 1) * T],
                    rhs=vk_sb[:, b * T : (b + 1) * T],
                    start=True, stop=True,
                )
            if j == 0:
                nc.vector.tensor_copy(out=g2sb, in_=gpart)
            else:
                nc.vector.tensor_add(out=g2sb, in0=g2sb, in1=gpart)
        nc.vector.tensor_scalar_mul(g2bf, g2sb, SCALE)
        if STOP <= 3:
            return

        # ============ v1-side projections ============
        for name, dst, scl in (
            ("k1", k1t_sb, 1.0),
            ("q1", q1t_sb, 1.0),
            ("wq1", v1qt_sb, SCALE),
            ("wk1", v1kt_sb, 1.0),
        ):
            for j in range(KT):
                wtile = wpool.tile([128, KT, 128], BF16, tag="w")
                nc.sync.dma_start(out=wtile, in_=wt_view[name][j])
                pj = ps_tv.tile([128, AP_ * R], F32, tag="tv")
                for kt in range(KT):
                    nc.tensor.matmul(
                        pj,
                        lhsT=wtile[:, kt, :],
                        rhs=v1t_sb[:, kt, :],
                        start=(kt == 0),
                        stop=(kt == KT - 1),
                    )
                if scl == 1.0:
                    nc.vector.tensor_copy(out=dst[:, j, :], in_=pj)
                else:
                    nc.scalar.mul(out=dst[:, j, :], in_=pj, mul=scl)
        if STOP <= 4:
            return

        # ============ G1, Gv1 ============
        g1ps = ps_tv.tile([R, AP_ * R], F32, tag="tv")
        for a in range(AP_):
            sl = slice(a * R, (a + 1) * R)
            for j in range(KT):
                nc.tensor.matmul(
                    g1ps[:, sl],
                    lhsT=v1qt_sb[:, j, sl],
                    rhs=v1kt_sb[:, j, sl],
                    start=(j == 0),
                    stop=(j == KT - 1),
                )
        nc.any.tensor_copy(out=g1_sb.rearrange("r a x -> r (a x)"), in_=g1ps)
        gv1ps = ps_tv.tile([R, AP_ * R], F32, tag="tv")
        for a in range(AP_):
            sl = slice(a * R, (a + 1) * R)
            for kt in range(KT):
                nc.tensor.matmul(
                    gv1ps[:, sl],
                    lhsT=v1t_sb[:, kt, sl],
                    rhs=v1t_sb[:, kt, sl],
                    start=(kt == 0),
                    stop=(kt == KT - 1),
                )
        nc.any.tensor_copy(out=gv1_sb.rearrange("r a x -> r (a x)"), in_=gv1ps)
        if STOP <= 5:
            return


        # ============ s1 -> p1 (batched per 4-b group g) ============
        for g in range(8):
            s1ps = ps_tv.tile([128, AP_ * R], F32, tag="tv")
            for kt in range(KT):
                nc.tensor.matmul(
                    s1ps,
                    lhsT=q2t_sb[:, kt, g * 128 : (g + 1) * 128],
                    rhs=k1t_sb[:, kt, :],
                    start=(kt == 0),
                    stop=(kt == KT - 1),
                )
            e1 = temps.tile([128, AP_, R], F32, tag="e128")
            nc.scalar.activation(out=e1, in_=s1ps.rearrange("p (a r) -> p a r", a=AP_), func=EXP, scale=SCALE)
            s1sum = sums.tile([128, AP_], F32, tag="s128")
            nc.vector.reduce_sum(out=s1sum, in_=e1, axis=AX)
            nc.vector.reciprocal(out=s1sum, in_=s1sum)
            nc.vector.tensor_tensor(
                p1_all[:, g], e1, s1sum[:, :, None].to_broadcast((128, AP_, R)),
                mybir.AluOpType.mult,
            )
        # base-0 per-pair blocks of p1 via partition-shift DMAs
        for bj in range(4):
            nc.sync.dma_start(out=p1v[:, :, bj, :, :], in_=p1_all[32 * bj : 32 * bj + 32, :, :, :])
        if STOP <= 7:
            return

        finals = ps_tv.tile([1, 3, 128], F32, tag="tv_fin", bufs=1)

        # ============ per-a pair stage ============
        for a in range(AP_):
            asl = slice(a * R, (a + 1) * R)

            # p1T for this a: [36 r, (b,t)=1024]
            p1t = temps.tile([R, 1024], BF16, tag="p1t")
            for g in range(8):
                tp = ps_tv.tile([128, 128], BF16, tag="tv")
                nc.tensor.transpose(tp[:R, :], p1_all[:, g, a, :], id128)
                nc.any.tensor_copy(out=p1t[:, g * 128 : (g + 1) * 128], in_=tp[:R, :])

            # s2 -> p2
            s2ps = ps_big.tile([R, 1024], F32, tag="big")
            for nh in range(2):
                for kt in range(KT):
                    nc.tensor.matmul(
                        s2ps[:, nh * 512 : (nh + 1) * 512],
                        lhsT=q1t_sb[:, kt, asl],
                        rhs=k2t_sb[:, kt, nh * 512 : (nh + 1) * 512],
                        start=(kt == 0),
                        stop=(kt == KT - 1),
                    )
            e2 = temps.tile([R, B2, T], F32, tag="e36")
            nc.vector.scalar_tensor_tensor(
                out=e2.rearrange("r b t -> r (b t)"), in0=s2ps, scalar=SCALE,
                in1=mb36, op0=mybir.AluOpType.mult, op1=mybir.AluOpType.add,
            )
            nc.scalar.activation(out=e2, in_=e2, func=EXP)
            ssum = sums.tile([R, B2], F32, tag="s36")
            nc.vector.reduce_sum(out=ssum, in_=e2, axis=AX)
            nc.vector.reciprocal(out=ssum, in_=ssum)
            p2 = temps.tile([R, 1024], BF16, tag="p2")
            nc.vector.tensor_tensor(
                p2.rearrange("r (b t) -> r b t", b=B2),
                e2, ssum[:, :, None].to_broadcast((R, B2, T)), mybir.AluOpType.mult,
            )

            if STOP <= 8:
                continue

            # p2T chunks -> base-0 per-pair blocks p2v [32 t, g, bj, 36 r]
            p2t = temps.tile([128, 8, R], BF16, tag="p2t")
            for g in range(8):
                tp = ps_tv.tile([128, 128], BF16, tag="tv")
                nc.tensor.transpose(tp[:, :R], p2[:, g * 128 : (g + 1) * 128], id128[0:R, 0:R])
                nc.any.tensor_copy(out=p2t[:, g, :], in_=tp[:, :R])
            p2v = temps.tile([T, 8, 4, R], BF16, tag="p2v")
            for bj in range(4):
                nc.sync.dma_start(out=p2v[:, :, bj, :], in_=p2t[32 * bj : 32 * bj + 32, :, :])

            # F2^T = G1[a] x p1T -> [36, 1024]
            f2tps = ps_big.tile([R, 1024], F32, tag="big")
            for nh in range(2):
                nc.tensor.matmul(
                    f2tps[:, nh * 512 : (nh + 1) * 512],
                    lhsT=g1_sb[:, a, :],
                    rhs=p1t[:, nh * 512 : (nh + 1) * 512],
                    start=True, stop=True,
                )
            f2t = temps.tile([R, 1024], BF16, tag="f2t")
            nc.any.tensor_copy(out=f2t, in_=f2tps)

            # sf2 per pair -> masked softmax -> P_f2
            sf2ps = ps_big.tile([T, 1024], F32, tag="big")
            for b in range(B2):
                bs = slice(b * T, (b + 1) * T)
                nc.tensor.matmul(sf2ps[:, bs], lhsT=f2t[:, bs], rhs=p1t[:, bs], start=True, stop=True)
            ef2 = temps.tile([T, B2, T], F32, tag="e36")
            nc.vector.tensor_add(out=ef2.rearrange("t b x -> t (b x)"), in0=sf2ps, in1=mb36[0:T, :])
            nc.scalar.activation(out=ef2, in_=ef2, func=EXP)
            fsum = sums.tile([T, B2], F32, tag="s36")
            nc.vector.reduce_sum(out=fsum, in_=ef2, axis=AX)
            nc.vector.reciprocal(out=fsum, in_=fsum)
            pf2 = temps.tile([T, 1024], BF16, tag="pf2")
            nc.vector.tensor_tensor(
                pf2.rearrange("t (b x) -> t b x", b=B2),
                ef2, fsum[:, :, None].to_broadcast((T, B2, T)), mybir.AluOpType.mult,
            )
            pf2t = temps.tile([128, 8, T], BF16, tag="pf2t")
            for g in range(8):
                tp = ps_tv.tile([128, 128], BF16, tag="tv")
                nc.tensor.transpose(tp[:, :T], pf2[:, g * 128 : (g + 1) * 128], id128[0:T, 0:T])
                nc.any.tensor_copy(out=pf2t[:, g, :], in_=tp[:, :T])
            pf2v = temps.tile([T, 8, 4, T], BF16, tag="pf2v")
            for bj in range(4):
                nc.sync.dma_start(out=pf2v[:, :, bj, :], in_=pf2t[32 * bj : 32 * bj + 32, :, :])

            if STOP <= 9:
                continue

            # Z2^T + u2 (all base 0)
            z2ps = ps_big.tile([R, 1024], F32, tag="big")
            for g in range(8):
                for bj in range(4):
                    b = 4 * g + bj
                    nc.tensor.matmul(
                        z2ps[:, b * T : (b + 1) * T],
                        lhsT=p1v[:, g, bj, a, :],
                        rhs=pf2v[:, g, bj, :],
                        start=True, stop=True,
                    )
            u2a = sums.tile([R, B2], F32, tag="u2")
            nc.vector.reduce_sum(out=u2a, in_=z2ps.rearrange("r (b t) -> r b t", b=B2), axis=AX)

            if STOP <= 10:
                continue

            # F1^T per pair into 64-padded halves -> f1t_sb [32 t', b, 36 i]
            f1t_sb = temps.tile([T, B2, R], BF16, tag="f1t")
            for half in range(2):
                fps = ps_big.tile([T, 16, 64], F32, tag="big")
                for loc in range(16):
                    b = 16 * half + loc
                    g, bj = b // 4, b % 4
                    nc.tensor.matmul(
                        fps[:, loc, 0:R],
                        lhsT=g2bf[:, b * T : (b + 1) * T],
                        rhs=p2v[:, g, bj, :],
                        start=True, stop=True,
                    )
                nc.any.tensor_copy(out=f1t_sb[:, 16 * half : 16 * half + 16, :], in_=fps[:, :, 0:R])
            # sf1 per pair -> softmax -> pf1 -> per-pair transposes
            pf1 = temps.tile([R, B2, 64], BF16, tag="pf1")
            for half in range(2):
                sf1ps = ps_big.tile([R, 16, 64], F32, tag="big")
                for loc in range(16):
                    b = 16 * half + loc
                    g, bj = b // 4, b % 4
                    nc.tensor.matmul(
                        sf1ps[:, loc, 0:R],
                        lhsT=f1t_sb[:, b, :],
                        rhs=p2v[:, g, bj, :],
                        start=True, stop=True,
                    )
                ef1 = temps.tile([R, 16, 64], F32, tag="e36")
                nc.scalar.activation(out=ef1[:, :, 0:R], in_=sf1ps[:, :, 0:R], func=EXP)
                hsum = sums.tile([R, 16], F32, tag="s36")
                nc.vector.reduce_sum(out=hsum, in_=ef1[:, :, 0:R], axis=AX)
                nc.vector.reciprocal(out=hsum, in_=hsum)
                nc.vector.tensor_tensor(
                    pf1[:, 16 * half : 16 * half + 16, 0:R],
                    ef1[:, :, 0:R], hsum[:, :, None].to_broadcast((R, 16, R)),
                    mybir.AluOpType.mult,
                )
            pf1t = temps.tile([R, B2, R], BF16, tag="pf1t")
            for q in range(8):
                tq = ps_tv.tile([R, 4, R], BF16, tag="tv")
                for k in range(4):
                    b = 4 * q + k
                    nc.tensor.transpose(tq[:, k, :], pf1[:, b, 0:R], id128[0:R, 0:R])
                nc.any.tensor_copy(out=pf1t[:, 4 * q : 4 * q + 4, :], in_=tq)

            if STOP <= 11:
                continue

            # Z1^T + u1 (64-padded halves)
            u1v = sums.tile([T, B2], F32, tag="u1")
            for half in range(2):
                z1ps = ps_big.tile([T, 16, 64], F32, tag="big")
                for loc in range(16):
                    b = 16 * half + loc
                    nc.tensor.matmul(
                        z1ps[:, loc, 0:R],
                        lhsT=p2[:, b * T : (b + 1) * T],
                        rhs=pf1t[:, b, :],
                        start=True, stop=True,
                    )
                nc.vector.reduce_sum(
                    out=u1v[:, 16 * half : 16 * half + 16], in_=z1ps[:, :, 0:R], axis=AX
                )

            if STOP <= 12:
                continue

            # ---- finals ----
            y_all = ps_tv.tile([T, B2], F32, tag="tv")
            g1_all = ps_tv.tile([T, B2], F32, tag="tv")
            for b in range(B2):
                nc.tensor.matmul(
                    y_all[:, b : b + 1],
                    lhsT=c_all[:, a, b * T : (b + 1) * T],
                    rhs=u2a[:, b : b + 1], start=True, stop=True,
                )
                nc.tensor.matmul(
                    g1_all[:, b : b + 1],
                    lhsT=gv2sb[:, b * T : (b + 1) * T],
                    rhs=u1v[:, b : b + 1], start=True, stop=True,
                )
            uy = smalls.tile([T, B2], F32, tag="uy")
            ug = smalls.tile([T, B2], F32, tag="ug")
            nc.vector.tensor_mul(out=uy, in0=u1v, in1=y_all)
            nc.vector.tensor_mul(out=ug, in0=u1v, in1=g1_all)
            g2ps = ps_tv.tile([R, B2], F32, tag="tv")
            nc.tensor.matmul(g2ps, lhsT=gv1_sb[:, a, :], rhs=u2a, start=True, stop=True)
            w2 = sums.tile([R, B2], F32, tag="w2")
            nc.vector.tensor_mul(out=w2, in0=u2a, in1=g2ps)
            asc = slice(a * 32, (a + 1) * 32)
            nc.tensor.matmul(finals[:, 0, asc], lhsT=ones32, rhs=uy, start=True, stop=True)
            nc.tensor.matmul(finals[:, 1, asc], lhsT=ones32, rhs=ug, start=True, stop=True)
            nc.tensor.matmul(finals[:, 2, asc], lhsT=ones36, rhs=w2, start=True, stop=True)

        if STOP <= 13:
            return
        # ============ finale ============
        sq1 = sums.tile([1, 128], F32, tag="fin1")
        sq2 = sums.tile([1, 128], F32, tag="fin2")
        nc.scalar.activation(out=sq1, in_=finals[:, 1, :], func=SQRT)
        nc.scalar.activation(out=sq2, in_=finals[:, 2, :], func=SQRT)
        nc.vector.tensor_scalar_add(sq1, sq1, EPS)
        nc.vector.tensor_scalar_add(sq2, sq2, EPS)
        nc.vector.tensor_mul(out=sq1, in0=sq1, in1=sq2)
        nc.vector.reciprocal(out=sq1, in_=sq1)
        out_sb = sums.tile([1, 128], F32, tag="fin3")
        nc.vector.tensor_mul(out=out_sb, in0=finals[:, 0, :], in1=sq1)
        nc.sync.dma_start(out=score_d, in_=out_sb)

    with tc.For_i(0, n_iters, 1):
        _iter_body()


def build_nc():
    from contextlib import ExitStack

    nc = bacc.Bacc("TRN2", target_bir_lowering=False, debug=False, num_devices=NCORES)
    v1t_d = nc.dram_tensor("v1t", [128, KT, AP_ * R], BF16, kind="ExternalInput").ap()
    v2t_d = nc.dram_tensor("v2t", [128, KT, B2 * T], BF16, kind="ExternalInput").ap()
    wts_d = {n: nc.dram_tensor(f"wt_{n}", [KT, 128, KT, 128], BF16, kind="ExternalInput").ap()
             for n in W_NAMES}
    mb_d = nc.dram_tensor("mb", [B2 * T], F32, kind="ExternalInput").ap()
    iters_d = nc.dram_tensor("iters", [1, 1], I32, kind="ExternalInput").ap()
    score_d = nc.dram_tensor("score", [1, AP_ * B2], F32, kind="ExternalOutput").ap()
    with tile.TileContext(nc) as tc:
        with ExitStack() as ctx:
            _body(nc, tc, ctx, (v1t_d, v2t_d, wts_d, mb_d, iters_d), score_d)
    nc.compile()
    return nc


_NC = None


def _get_nc():
    global _NC
    if _NC is None:
        _NC = build_nc()
    return _NC


def make_in_maps(inputs, iters=1):
    v1 = np.asarray(inputs["v1"], np.float32)
    v2 = np.asarray(inputs["v2"], np.float32)
    mask = np.asarray(inputs["mask"])
    mbias = np.where(mask == 0, np.float32(NEG), np.float32(0.0)).reshape(-1)
    # [d, x] -> [p, kt, x] contiguous (partition-contiguous DMA)
    v2t = np.ascontiguousarray(
        v2.reshape(B2 * T, D).T.reshape(KT, 128, B2 * T).transpose(1, 0, 2)
    ).astype(ml_dtypes.bfloat16)
    wmap = {"k1": "Wk1", "q1": "Wq1", "wq1": "Wwq1", "wk1": "Wwk1",
            "k2": "Wk2", "q2": "Wq2", "wq2": "Wwq2", "wk2": "Wwk2"}
    def _wprep(w):
        # W.T [d, n] -> [jt, p, kt, nn] so each [128, KT, 128] j-tile is
        # partition-contiguous in DRAM
        wt = np.asarray(w, np.float32).T.reshape(KT, 128, KT, 128).transpose(2, 1, 0, 3)
        return np.ascontiguousarray(wt).astype(ml_dtypes.bfloat16)

    wts = {n: _wprep(inputs[k]) for n, k in wmap.items()}
    in_maps = []
    for c in range(NCORES):
        v1c = v1[c * AP_ : (c + 1) * AP_].reshape(AP_ * R, D)
        v1tc = np.ascontiguousarray(
            v1c.T.reshape(KT, 128, AP_ * R).transpose(1, 0, 2)
        ).astype(ml_dtypes.bfloat16)
        m = {"v1t": v1tc,
             "v2t": v2t, "mb": mbias,
             "iters": np.array([[iters]], np.int32)}
        for n, w in wts.items():
            m[f"wt_{n}"] = w
        in_maps.append(m)
    return in_maps


def run_on_device(in_maps):
    nc = _get_nc()
    res = run_bass_kernel_spmd(nc, in_maps, core_ids=list(range(NCORES)))
    return np.concatenate(
        [res.results[c]["score"].reshape(AP_, B2) for c in range(NCORES)], axis=0
    )


def kernel(**inputs) -> np.ndarray:
    return run_on_device(make_in_maps(inputs, iters=1)).astype(np.float32)
